# revision 62
# baseline (speedup 1.0000x reference)
"""GAT (2-layer, 4-head) + MLP/BatchNorm predictor on 8 Trainium2 NeuronCores.

v2 strategy (graph-parallel, dst-sharded):
  - Nodes split contiguously: core c owns dsts [c*6250, (c+1)*6250). Edges live
    with their dst core, sorted by dst, grouped into 49 chunks of <=128 dsts.
  - Features kept TRANSPOSED host-side (featT [F, N]) so projections need no
    on-device transposes: lhsT tiles are direct slices.
  - Per layer, every core computes the FULL projection table (redundant
    compute, zero communication). Table row n = y(n) = 256 fp16 (512 B), where
    y = h @ M per head with M an orthogonal-ish basis whose first column is
    al: el(n,h) = y[n, h*64] comes free from the gather; the basis is undone
    after aggregation by a tiny Minv matmul in transposed space.
  - Edge phase per chunk: dma_gather of src rows (512B each, two gathers for
    the int16 index split), er per slot via precomputed fp8 transposed-mask
    matmuls, attention ex = exp(lrelu(el+er)) on the Act engine, features
    scaled by ex on DVE, scatter-aggregate + softmax denominators via
    precomputed fp8 forward-mask matmuls accumulating in PSUM; division in
    node space; epilogue (Minv, bias, relu, head-mean) in transposed space so
    the layer output lands directly in [D, nodes] layout.
  - Collectives: AllGather of x2sliceT (fp16) for the layer-2 table build;
    single AllReduce of BatchNorm statistics (var = E[z^2] - mu^2).
"""
import sys

sys.path.insert(0, "/opt/trn_rl_repo")

import numpy as np

N = 50000
F_IN = 128
H = 4
D = 64
HD = 256
NCORES = 8
NSHARD = N // NCORES          # 6250
P = 128
NCHUNK = (NSHARD + P - 1) // P  # 49 (last chunk 106 dsts)
SPLIT = 32768                 # int16 gather index limit
GCOL = 1024                   # projection column-group size


def configure(n, split=32768):
    """Override problem size (for simulator debugging)."""
    global N, NSHARD, NCHUNK, SPLIT
    N = n
    NSHARD = N // NCORES
    NCHUNK = (NSHARD + P - 1) // P
    SPLIT = split


CA_CH = 26                    # chunks in AllGather half A
MLP_H = 200
NCLS = 2
NEG = 0.2
EPS = 1e-5
TBL = 256                     # fp16 elems per table row (512 B)


# ----------------------------------------------------------------------------
# Host-side preprocessing
# ----------------------------------------------------------------------------

def _head_basis(al):
    """al [D] -> (M [D, D] f64 with M[:,0] = al, other cols orthonormal;
    Minv [D, D] f64)."""
    al = np.asarray(al, np.float64)
    nrm = np.linalg.norm(al)
    if nrm < 1e-12:
        return np.eye(D), np.eye(D)
    a = al / nrm
    v = a.copy()
    v[0] -= 1.0
    vv = v @ v
    if vv < 1e-20:
        Hh = np.eye(D)
    else:
        Hh = np.eye(D) - 2.0 * np.outer(v, v) / vv   # H @ e0 = a, H = H^-1
    M = Hh * np.concatenate([[nrm], np.ones(D - 1)])[None, :]   # scale col 0
    Minv = Hh / np.concatenate([[nrm], np.ones(D - 1)])[:, None]  # scale row 0
    return M, Minv


def _fold_weights(W, al, ar):
    """W [F, H*D], al/ar [H, D] ->
    (Wy [F, H*D] f16, Wer [F, H] f16, Minv [D, H*D] f16 per-head concat)."""
    F = W.shape[0]
    W64 = np.asarray(W, np.float64).reshape(F, H, D)
    Wy = np.zeros((F, H, D), np.float64)
    Minv = np.zeros((D, H, D), np.float64)
    for h in range(H):
        M, Mi = _head_basis(al[h])
        Wy[:, h, :] = W64[:, h, :] @ M
        Minv[:, h, :] = Mi
    Wer = np.einsum('fhd,hd->fh', W64, np.asarray(ar, np.float64))
    return (Wy.reshape(F, H * D).astype(np.float16),
            Wer.astype(np.float16),
            Minv.reshape(D, H * D).astype(np.float16))


def _prep_edges(src, dst, f8):
    """Build per-core gather/mask arrays.

    Returns (plan, per-core list of (IDX [128, 8*totT] i16,
    M8 [128, totT*128] f8 forward mask, MT8 [128, totT*128] f8 transposed
    mask))."""
    src = np.asarray(src)
    dst = np.asarray(dst)
    per_core = []
    for c in range(NCORES):
        m = (dst >= c * NSHARD) & (dst < (c + 1) * NSHARD)
        es, ed = src[m], dst[m] - c * NSHARD
        order = np.argsort(ed, kind="stable")
        es, ed = es[order], ed[order]
        starts = np.searchsorted(ed, np.arange(0, NCHUNK * P, P))
        ends = np.searchsorted(ed, np.minimum(np.arange(P, (NCHUNK + 1) * P, P), NSHARD))
        chunks = []
        for j in range(NCHUNK):
            cs, ce = starts[j], ends[j]
            s_j, d_j = es[cs:ce], ed[cs:ce] - j * P
            lo = s_j < SPLIT
            chunks.append((s_j[lo], d_j[lo], s_j[~lo] - SPLIT, d_j[~lo]))
        per_core.append(chunks)

    T_lo = np.zeros(NCHUNK, np.int64)
    T_hi = np.zeros(NCHUNK, np.int64)
    for c in range(NCORES):
        for j in range(NCHUNK):
            slo, _, shi, _ = per_core[c][j]
            T_lo[j] = max(T_lo[j], -(-len(slo) // P))
            T_hi[j] = max(T_hi[j], -(-len(shi) // P))
    T_lo = np.maximum(T_lo, 1)
    totT = int((T_lo + T_hi).sum())

    def wrap_idx(flat):
        """dma_gather index layout: idx j at [16*rep + j%16, j//16], rep 0..7."""
        n = len(flat)
        cols = n // 16
        a = flat.reshape(cols, 16).T.astype(np.int16)
        return np.tile(a, (8, 1))

    iota = np.arange(P)
    arrays = []
    for c in range(NCORES):
        idx_cols = []
        m8 = np.zeros((P, totT, P), np.float16)  # [slot_p, slot_t, dst]
        mt8 = np.zeros((P, totT, P), f8)         # [dst, slot_t, slot_p]
        t0 = 0

        def emit_masks(s_j, d_j, T):
            nonlocal t0
            nslot = int(T) * P
            idx = np.zeros(nslot, np.int16)
            dl = np.full(nslot, -1, np.int64)
            idx[: len(s_j)] = s_j
            dl[: len(s_j)] = d_j
            dlm = dl.reshape(int(T), P)                 # [t, p]
            m8[:, t0:t0 + int(T), :] = (
                dlm.T[:, :, None] == iota[None, None, :])
            mt8[:, t0:t0 + int(T), :] = (
                dlm[None, :, :] == iota[:, None, None])
            t0 += int(T)
            return idx

        # pair layout: per pair (j, j+1): lo_j | lo_j+1 | hi_j | hi_j+1
        for k in range(0, NCHUNK, 2):
            js = [k] + ([k + 1] if k + 1 < NCHUNK else [])
            lo_parts, hi_parts = [], []
            for j in js:
                slo, dlo, shi, dhi = per_core[c][j]
                lo_parts.append(emit_masks(slo, dlo, T_lo[j]))
            for j in js:
                slo, dlo, shi, dhi = per_core[c][j]
                if T_hi[j]:
                    hi_parts.append(emit_masks(shi, dhi, T_hi[j]))
            idx_cols.append(wrap_idx(np.concatenate(lo_parts)))
            if hi_parts:
                idx_cols.append(wrap_idx(np.concatenate(hi_parts)))
        assert t0 == totT
        IDX = np.concatenate(idx_cols, axis=1)
        assert IDX.shape == (P, 8 * totT)
        arrays.append((IDX, m8.reshape(P, totT * P), mt8.reshape(P, totT * P)))

    plan = {"T_lo": T_lo.tolist(), "T_hi": T_hi.tolist(), "totT": totT}
    return plan, arrays


# ----------------------------------------------------------------------------
# Bass program
# ----------------------------------------------------------------------------

def build_nc(plan, phases='full', max_chunks=None, reps=1):
    import concourse.bacc as bacc
    import concourse.bass as bass
    import concourse.tile as tile
    from concourse import mybir

    FP16 = mybir.dt.float16
    F32 = mybir.dt.float32
    F8 = mybir.dt.float8e4
    I16 = mybir.dt.int16
    ALU = mybir.AluOpType
    ACTF = mybir.ActivationFunctionType

    T_lo, T_hi, totT = plan["T_lo"], plan["T_hi"], plan["totT"]

    nc = bacc.Bacc("TRN2", target_bir_lowering=False, debug=False,
                   num_devices=NCORES)

    dp = lambda name, shape, dt: nc.declare_dram_parameter(name, shape, dt, isOutput=False)
    featT = dp("featT", [F_IN, N], FP16)
    fownT = dp("fownT", [F_IN, NSHARD], FP16)
    IDX = dp("IDX", [P, 8 * totT], I16)
    M16 = dp("M16", [P, totT * P], FP16)
    MT8 = dp("MT8", [P, totT * P], F8)
    IDENT = dp("IDENT", [P, P], FP16)
    WY1 = dp("WY1", [F_IN, HD], FP16)
    WER1 = dp("WER1", [F_IN, H], FP16)
    WY2 = dp("WY2", [D, HD], FP16)
    WER2 = dp("WER2", [D, H], FP16)
    MINV1 = dp("MINV1", [D, H * D], FP16)   # per-head Minv, concatenated
    MINV2 = dp("MINV2", [D, H * D], FP16)
    BIAS = dp("BIAS", [D, 2 * H], F32)   # col lay*4+h = b[lay][h,:] * 0.25
    WM1B = dp("WM1B", [D + 1, MLP_H], F32)   # row D = bm1
    GB = dp("GB", [P, 4], F32)           # gamma c1, gamma c2, beta c1, beta c2
    WM2C1F = dp("WM2C1F", [P, NCLS], F32)
    WM2C2F = dp("WM2C2F", [MLP_H - P, NCLS], F32)
    BM2 = dp("BM2", [1, NCLS], F32)
    ONESF = dp("ONESF", [P, 1], F32)
    ONESH = dp("ONESH", [P, 1], FP16)

    out = nc.declare_dram_parameter("out", [NSHARD, NCLS], F32, isOutput=True)

    table1 = nc.dram_tensor("table1", [N, TBL], FP16)
    table2 = nc.dram_tensor("table2", [N, TBL], FP16)
    CA = min(CA_CH * P, NSHARD)
    CB = NSHARD - CA
    x2aT = nc.dram_tensor("x2aT", [D, CA], FP16)
    x2fullTa = nc.dram_tensor("x2fullTa", [NCORES * D, CA], FP16,
                              addr_space="Shared")
    if CB:
        x2bT = nc.dram_tensor("x2bT", [D, CB], FP16)
        x2fullTb = nc.dram_tensor("x2fullTb", [NCORES * D, CB], FP16,
                                  addr_space="Shared")
    ccin = nc.dram_tensor("ccin", [P, 4], F32)
    ccout = nc.dram_tensor("ccout", [P, 4], F32, addr_space="Shared")
    ccc = nc.dram_tensor("ccc", [1, NCLS], F32)

    def mk_ap(base_ap, offset_elems, ap):
        return bass.AP(tensor=base_ap.tensor,
                       offset=base_ap.offset + offset_elems, ap=ap)

    class _SkipRest(Exception):
        pass

    with tile.TileContext(nc) as tc:
        import contextlib
        try:
          with contextlib.ExitStack() as ctx:
            singles = ctx.enter_context(tc.tile_pool(name="singles", bufs=1))

            def load_const(param, shape, dtype, tag):
                t = singles.tile(shape, dtype, tag=tag)
                nc.sync.dma_start(out=t[:], in_=param[:])
                return t

            ident = load_const(IDENT, [P, P], FP16, "c_ident")
            wy1 = load_const(WY1, [F_IN, HD], FP16, "c_wy1")
            wer1 = load_const(WER1, [F_IN, H], FP16, "c_wer1")
            wy2 = load_const(WY2, [D, HD], FP16, "c_wy2")
            wer2 = load_const(WER2, [D, H], FP16, "c_wer2")
            minv1 = load_const(MINV1, [D, H * D], FP16, "c_minv1")
            minv2 = load_const(MINV2, [D, H * D], FP16, "c_minv2")
            biasc = load_const(BIAS, [D, 2 * H], F32, "c_bias")
            wm1b = load_const(WM1B, [D + 1, MLP_H], F32, "c_wm1b")
            gb = load_const(GB, [P, 4], F32, "c_gb")
            wm2c1f = load_const(WM2C1F, [P, NCLS], F32, "c_wm2c1f")
            wm2c2f = load_const(WM2C2F, [MLP_H - P, NCLS], F32, "c_wm2c2f")
            bm2 = load_const(BM2, [1, NCLS], F32, "c_bm2")
            onesf = load_const(ONESF, [P, 1], F32, "c_onesf")
            onesh = load_const(ONESH, [P, 1], FP16, "c_onesh")

            def _run_once(rep):
                sfx = f"_r{rep}" if reps > 1 else ""
                ero1 = singles.tile([P, NCHUNK, H], FP16, tag="c_ero1")
                ero2 = singles.tile([P, NCHUNK, H], FP16, tag="c_ero2")
                # x3aug: rows 0..63 = x3T (f32), row 64 = ones
                x3aug = singles.tile([D + 1, NCHUNK, P], F32, tag="c_x3aug")

                # ------------- projection: table rows via direct lhsT -------
                def proj_cols(src_tile, scol, ncols, wy, table, row0, sb, ps,
                              alt):
                    """Project ncols nodes (from src_tile cols scol..) into
                    table rows row0.. ; batched out-DMA."""
                    nsub = (ncols + P - 1) // P
                    rowt = sb.tile([P, 8, TBL], FP16, tag="rowt")
                    for i in range(nsub):
                        r = min(P, ncols - i * P)
                        hp = ps.tile([P, TBL], F32, tag="hp")
                        nc.tensor.matmul(hp[:r, :],
                                         lhsT=src_tile[:, scol + i * P: scol + i * P + r],
                                         rhs=wy[:], start=True, stop=True)
                        eng = nc.vector if (alt[0] % 2 == 0) else nc.scalar
                        alt[0] += 1
                        if eng is nc.vector:
                            nc.vector.tensor_copy(out=rowt[:r, i, :], in_=hp[:r, :])
                        else:
                            nc.scalar.copy(out=rowt[:r, i, :], in_=hp[:r, :])
                    kfull = ncols // P
                    rt = ncols - kfull * P
                    tap = table[:]
                    if kfull:
                        dst = mk_ap(tap, row0 * TBL,
                                    [[TBL, P], [P * TBL, kfull], [1, TBL]])
                        nc.gpsimd.dma_start(out=dst, in_=rowt[:, 0:kfull, :])
                    if rt:
                        dst = mk_ap(tap, (row0 + kfull * P) * TBL,
                                    [[TBL, rt], [1, TBL]])
                        nc.gpsimd.dma_start(out=dst, in_=rowt[:rt, kfull, :])

                def projection1():
                    alt = [0]
                    with tc.tile_pool(name="p1sb", bufs=3) as sb, \
                         tc.tile_pool(name="p1ps", bufs=3, space="PSUM") as ps:
                        ng = (N + GCOL - 1) // GCOL
                        for g in range(ng):
                            c0 = g * GCOL
                            cols = min(GCOL, N - c0)
                            xt = sb.tile([F_IN, GCOL], FP16, tag="xt")
                            nc.gpsimd.dma_start(out=xt[:, :cols],
                                                in_=featT[:, c0:c0 + cols])
                            proj_cols(xt, 0, cols, wy1, table1, c0, sb, ps, alt)

                def er_prologue(xT_tile, wer, ero, sb, ps):
                    for j in range(NCHUNK):
                        r = min(P, NSHARD - j * P)
                        ep = ps.tile([P, H], F32, tag="ep")
                        nc.tensor.matmul(ep[:r, :],
                                         lhsT=xT_tile[:, j * P:j * P + r],
                                         rhs=wer[:], start=True, stop=True)
                        nc.scalar.copy(out=ero[:r, j, :], in_=ep[:r, :])

                def er_prologue1():
                    with tc.tile_pool(name="e1sb", bufs=1) as sb, \
                         tc.tile_pool(name="e1ps", bufs=2, space="PSUM") as ps:
                        fo = sb.tile([F_IN, NSHARD], FP16, tag="fo")
                        nc.gpsimd.dma_start(out=fo[:], in_=fownT[:])
                        er_prologue(fo, wer1, ero1, sb, ps)

                def projection2():
                    alt = [0]
                    with tc.tile_pool(name="p2sb", bufs=2) as sb, \
                         tc.tile_pool(name="p2ps", bufs=3, space="PSUM") as ps:
                        # er prologue for layer 2 from own slice (half A)
                        sx2a = sb.tile([D, CA], FP16, tag="sx2a")
                        nc.gpsimd.dma_start(out=sx2a[:], in_=x2aT[:])
                        for j in range(CA // P):
                            r = min(P, NSHARD - j * P)
                            ep = ps.tile([P, H], F32, tag="ep")
                            nc.tensor.matmul(ep[:r, :],
                                             lhsT=sx2a[:, j * P:j * P + r],
                                             rhs=wer2[:], start=True, stop=True)
                            nc.scalar.copy(out=ero2[:r, j, :], in_=ep[:r, :])
                        # half-A projections for every block
                        for c in range(NCORES):
                            bxa = sb.tile([D, CA], FP16, tag="bxa")
                            nc.gpsimd.dma_start(
                                out=bxa[:], in_=x2fullTa[c * D:(c + 1) * D, :])
                            ng = (CA + GCOL - 1) // GCOL
                            for g in range(ng):
                                c0 = g * GCOL
                                cols = min(GCOL, CA - c0)
                                proj_cols(bxa, c0, cols, wy2, table2,
                                          c * NSHARD + c0, sb, ps, alt)
                        if not CB:
                            return
                        # er prologue half B
                        sx2b = sb.tile([D, CB], FP16, tag="sx2b")
                        nc.gpsimd.dma_start(out=sx2b[:], in_=x2bT[:])
                        for j in range(CA // P, NCHUNK):
                            r = min(P, NSHARD - j * P)
                            c0 = j * P - CA
                            ep = ps.tile([P, H], F32, tag="ep")
                            nc.tensor.matmul(ep[:r, :],
                                             lhsT=sx2b[:, c0:c0 + r],
                                             rhs=wer2[:], start=True, stop=True)
                            nc.scalar.copy(out=ero2[:r, j, :], in_=ep[:r, :])
                        # half-B projections
                        for c in range(NCORES):
                            bxb = sb.tile([D, CB], FP16, tag="bxb")
                            nc.gpsimd.dma_start(
                                out=bxb[:], in_=x2fullTb[c * D:(c + 1) * D, :])
                            ng = (CB + GCOL - 1) // GCOL
                            for g in range(ng):
                                c0 = g * GCOL
                                cols = min(GCOL, CB - c0)
                                proj_cols(bxb, c0, cols, wy2, table2,
                                          c * NSHARD + CA + c0, sb, ps, alt)

                # ------------------------- edge phase ---------------------------
                def edge_phase(table, ero, minv, bias_sl, lay, hooks=False):
                    nch = NCHUNK if max_chunks is None else min(max_chunks, NCHUNK)
                    epsz_bufs = 1 if hooks else 2
                    with tc.tile_pool(name="eg", bufs=2) as eg, \
                         tc.tile_pool(name="em", bufs=2) as em, \
                         tc.tile_pool(name="es", bufs=2) as es_pool, \
                         tc.tile_pool(name="eps", bufs=2, space="PSUM") as eps, \
                         tc.tile_pool(name="epsa", bufs=2, space="PSUM") as epsa, \
                         tc.tile_pool(name="epsz", bufs=epsz_bufs, space="PSUM") as epsz, \
                         tc.tile_pool(name="ezp", bufs=1, space="PSUM") as ezp:
                        if hooks:
                            acc_sb = singles.tile([P, 4], F32, tag="c_accsb")
                            nc.vector.memset(acc_sb[:], 0.0)
                        toff = 0
                        for k in range(0, nch, 2):
                            js = [k] + ([k + 1] if k + 1 < nch else [])
                            Tls = [T_lo[j] for j in js]
                            Ths = [T_hi[j] for j in js]
                            Tlp, Thp = sum(Tls), sum(Ths)
                            Tp = Tlp + Thp
                            # per-chunk tile index lists in the pair buffer
                            tiles_of = {}
                            o = 0
                            for j, tl in zip(js, Tls):
                                tiles_of[j] = list(range(o, o + tl))
                                o += tl
                            for j, th in zip(js, Ths):
                                tiles_of[j] += list(range(o, o + th))
                                o += th
                            idxt = es_pool.tile([P, 8 * Tp], I16, tag="idxt")
                            nc.sync.dma_start(out=idxt[:], in_=IDX[:, 8 * toff:8 * (toff + Tp)])
                            m16 = em.tile([P, Tp, P], FP16, tag="m16")
                            nc.sync.dma_start(out=m16[:], in_=M16[:, toff * P:(toff + Tp) * P])
                            mt8 = em.tile([P, Tp * P], F8, tag="mt8")
                            nc.sync.dma_start(out=mt8[:], in_=MT8[:, toff * P:(toff + Tp) * P])
                            gbuf = eg.tile([P, Tp, TBL], FP16, tag="gbuf")
                            nc.gpsimd.dma_gather(
                                out_ap=gbuf[:, 0:Tlp, :], in_ap=table[0:min(SPLIT, N), :],
                                idxs_ap=idxt[:, 0:8 * Tlp], num_idxs=P * Tlp,
                                num_idxs_reg=P * Tlp, elem_size=TBL, single_packet=False)
                            if Thp:
                                nc.gpsimd.dma_gather(
                                    out_ap=gbuf[:, Tlp:Tp, :], in_ap=table[SPLIT:N, :],
                                    idxs_ap=idxt[:, 8 * Tlp:8 * Tp], num_idxs=P * Thp,
                                    num_idxs_reg=P * Thp, elem_size=TBL, single_packet=False)
                            # er per slot: fp8-mask matmuls into one PSUM strip
                            erps = eps.tile([P, Tp, H], F32, tag="erp")
                            for j in js:
                                for t in tiles_of[j]:
                                    nc.tensor.matmul(erps[:, t, :],
                                                     lhsT=mt8[:, t * P:(t + 1) * P],
                                                     rhs=ero[:, j, :], start=True, stop=True)
                            # e = el + er ; el = gbuf[:, :, h*64] strided view
                            gap = gbuf[:]
                            el_view = mk_ap(gap, 0, [gap.ap[0], [TBL, Tp], [D, H]])
                            e_sb = es_pool.tile([P, Tp, H], F32, tag="e_sb")
                            nc.vector.tensor_tensor(out=e_sb[:], in0=el_view,
                                                    in1=erps[:], op=ALU.add)
                            lr = es_pool.tile([P, Tp, H], F32, tag="lr")
                            nc.vector.tensor_scalar(out=lr[:], in0=e_sb[:],
                                                    scalar1=NEG, scalar2=None,
                                                    op0=ALU.mult)
                            nc.vector.tensor_tensor(out=lr[:], in0=e_sb[:],
                                                    in1=lr[:], op=ALU.max)
                            gbuf2 = eg.tile([P, Tp, TBL + H], FP16, tag="gbuf2")
                            nc.scalar.activation(gbuf2[:, :, TBL:TBL + H], lr[:],
                                                 ACTF.Exp)
                            # scale gathered features by ex (broadcast over d)
                            g2 = gbuf2[:]
                            ex_b = mk_ap(g2, TBL,
                                         [g2.ap[0], [TBL + H, Tp], [1, H], [0, D]])
                            feat_o = mk_ap(g2, 0,
                                           [g2.ap[0], [TBL + H, Tp], [D, H], [1, D]])
                            feat_i = mk_ap(gap, 0,
                                           [gap.ap[0], [TBL, Tp], [D, H], [1, D]])
                            nc.vector.tensor_tensor(out=feat_o, in0=feat_i,
                                                    in1=ex_b, op=ALU.mult)
                            for j in js:
                                rows = min(P, NSHARD - j * P)
                                # aggregate: fp16-mask matmuls accumulating in PSUM
                                agg = epsa.tile([P, TBL + H], F32, tag="agg")
                                tl = tiles_of[j]
                                for i, t in enumerate(tl):
                                    nc.tensor.matmul(agg[:], lhsT=m16[:, t, :],
                                                     rhs=gbuf2[:, t, :],
                                                     start=(i == 0),
                                                     stop=(i == len(tl) - 1))
                                s_sb = es_pool.tile([P, H], F32, tag="s_sb")
                                nc.vector.tensor_scalar(out=s_sb[:],
                                                        in0=agg[:, TBL:TBL + H],
                                                        scalar1=1e-30, scalar2=None,
                                                        op0=ALU.max)
                                sr = es_pool.tile([P, H], F32, tag="sr")
                                nc.vector.reciprocal(sr[:], s_sb[:])
                                aggc = es_pool.tile([P, HD], FP16, tag="aggc")
                                for h in range(H):
                                    nc.scalar.activation(
                                        aggc[:, h * D:(h + 1) * D],
                                        agg[:, h * D:(h + 1) * D], ACTF.Copy,
                                        scale=sr[:, h:h + 1])
                                # per-head transpose, Minv + bias, relu, head-mean
                                tp4 = epsz.tile([D, H, P], FP16, tag="tp4")
                                for h in range(H):
                                    nc.tensor.transpose(
                                        out=tp4[:, h, :],
                                        in_=aggc[:, h * D:(h + 1) * D],
                                        identity=ident[:])
                                aT4 = es_pool.tile([D, H, P], FP16, tag="aT4")
                                nc.scalar.copy(out=aT4[:], in_=tp4[:])
                                z4 = epsz.tile([D, H, P], F32, tag="z4")
                                for h in range(H):
                                    nc.tensor.matmul(z4[:, h, :],
                                                     lhsT=minv[:, h * D:(h + 1) * D],
                                                     rhs=aT4[:, h, :],
                                                     start=True, stop=True)
                                z4r = es_pool.tile(
                                    [D, H, P], FP16 if lay == 1 else F32, tag="z4r")
                                for h in range(H):
                                    nc.scalar.activation(
                                        z4r[:, h, :], z4[:, h, :], ACTF.Relu,
                                        bias=biasc[:, bias_sl + h:bias_sl + h + 1],
                                        scale=0.25)
                                h1 = es_pool.tile([D, P], F32, tag="h1")
                                nc.vector.tensor_tensor(out=h1[:], in0=z4r[:, 0, :],
                                                        in1=z4r[:, 1, :], op=ALU.add)
                                h2 = es_pool.tile([D, P], F32, tag="h2")
                                nc.vector.tensor_tensor(out=h2[:], in0=z4r[:, 2, :],
                                                        in1=z4r[:, 3, :], op=ALU.add)
                                if lay == 1:
                                    xoT = es_pool.tile([D, P], FP16, tag="xoT")
                                    nc.vector.tensor_tensor(out=xoT[:], in0=h1[:],
                                                            in1=h2[:], op=ALU.add)
                                    if j * P < CA:
                                        nc.sync.dma_start(
                                            out=x2aT[:, j * P:j * P + rows],
                                            in_=xoT[:, :rows])
                                    else:
                                        c0 = j * P - CA
                                        nc.sync.dma_start(
                                            out=x2bT[:, c0:c0 + rows],
                                            in_=xoT[:, :rows])
                                else:
                                    nc.vector.tensor_tensor(out=x3aug[0:D, j, :],
                                                            in0=h1[:], in1=h2[:],
                                                            op=ALU.add)
                                    if hooks:
                                        zp = ezp.tile([P, MLP_H], F32, tag="zp")
                                        nc.tensor.matmul(zp[:], lhsT=x3aug[:, j, :],
                                                         rhs=wm1b[:], start=True,
                                                         stop=True)
                                        zsb = es_pool.tile([P, MLP_H], F32, tag="zsb")
                                        if rows < P:
                                            nc.vector.memset(zsb[:], 0.0)
                                        nc.scalar.activation(zsb[:rows], zp[:rows],
                                                             ACTF.Relu)
                                        zq = es_pool.tile([P, MLP_H], F32, tag="zq")
                                        nc.vector.tensor_tensor(out=zq[:], in0=zsb[:],
                                                                in1=zsb[:],
                                                                op=ALU.mult)
                                        strip = ezp.tile([P, 4], F32, tag="strip")
                                        nc.tensor.matmul(strip[:, 0:1],
                                                         lhsT=zsb[:, 0:P],
                                                         rhs=onesf[:],
                                                         start=True, stop=True)
                                        nc.tensor.matmul(strip[:, 1:2],
                                                         lhsT=zq[:, 0:P],
                                                         rhs=onesf[:],
                                                         start=True, stop=True)
                                        nc.tensor.matmul(strip[:MLP_H - P, 2:3],
                                                         lhsT=zsb[:, P:MLP_H],
                                                         rhs=onesf[:],
                                                         start=True, stop=True)
                                        nc.tensor.matmul(strip[:MLP_H - P, 3:4],
                                                         lhsT=zq[:, P:MLP_H],
                                                         rhs=onesf[:],
                                                         start=True, stop=True)
                                        nc.vector.tensor_tensor(
                                            out=acc_sb[:, 0:2], in0=acc_sb[:, 0:2],
                                            in1=strip[:, 0:2], op=ALU.add)
                                        nc.vector.tensor_tensor(
                                            out=acc_sb[:MLP_H - P, 2:4],
                                            in0=acc_sb[:MLP_H - P, 2:4],
                                            in1=strip[:MLP_H - P, 2:4], op=ALU.add)
                            toff += Tp
                        if hooks:
                            pk = singles.tile([P, 4], F32, tag="c_pk")
                            nc.vector.memset(pk[:], 0.0)
                            nc.vector.tensor_copy(out=pk[:, 0:1], in_=acc_sb[:, 0:1])
                            nc.vector.tensor_copy(out=pk[:MLP_H - P, 1:2],
                                                  in_=acc_sb[:MLP_H - P, 2:3])
                            nc.vector.tensor_copy(out=pk[:, 2:3], in_=acc_sb[:, 1:2])
                            nc.vector.tensor_copy(out=pk[:MLP_H - P, 3:4],
                                                  in_=acc_sb[:MLP_H - P, 3:4])
                            nc.sync.dma_start(out=ccin[:], in_=pk[:])

                # ------------------------------ go ------------------------------
                order = ["P1", "E1", "AG", "P2", "E2", "full"]
                upto = order.index(phases if phases != "full" else "full")
                done = False

                projection1()
                er_prologue1()
                done = upto <= order.index("P1")
                if not done:
                    edge_phase(table1, ero1, minv1, 0, lay=1)
                    done = upto <= order.index("E1")
                if not done:
                    nc.gpsimd.collective_compute(
                        "AllGather", mybir.AluOpType.bypass,
                        replica_groups=[list(range(NCORES))],
                        ins=[x2aT[:]], outs=[x2fullTa[:]])
                    if CB:
                        nc.gpsimd.collective_compute(
                            "AllGather", mybir.AluOpType.bypass,
                            replica_groups=[list(range(NCORES))],
                            ins=[x2bT[:]], outs=[x2fullTb[:]])
                    done = upto <= order.index("AG")
                if not done:
                    projection2()
                    nc.vector.memset(x3aug[D:D + 1, :, :], 1.0)
                    edge_phase(table2, ero2, minv2, H, lay=2,
                               hooks=(upto > order.index("E2")))
                    done = upto <= order.index("E2")
                skip_mlp = done
                if skip_mlp:
                    with tc.tile_pool(name="dbg0", bufs=1) as dbg0:
                        z = dbg0.tile([P, NCLS], F32, tag="dbgz")
                        nc.vector.memset(z[:], 0.0)
                        for j in range(NCHUNK):
                            r0 = j * P
                            rows = min(P, NSHARD - r0)
                            nc.sync.dma_start(out=out[r0:r0 + rows, :], in_=z[:rows])
                    raise _SkipRest()

                # ---------------- MLP (pass A ran inside E2) --------------------
                nc.gpsimd.collective_compute(
                    "AllReduce", mybir.AluOpType.add,
                    replica_groups=[list(range(NCORES))],
                    ins=[ccin[:]], outs=[ccout[:]])

                # pass C: BN constants + folded final weights
                with tc.tile_pool(name="m2", bufs=1) as m2, \
                     tc.tile_pool(name="m2p", bufs=2, space="PSUM") as m2p:
                    stg = m2.tile([P, 4], F32, tag="stg")
                    nc.sync.dma_start(out=stg[:], in_=ccout[:])
                    mu = m2.tile([P, 2], F32, tag="mu")
                    nc.vector.tensor_scalar(out=mu[:], in0=stg[:, 0:2],
                                            scalar1=1.0 / N, scalar2=None,
                                            op0=ALU.mult)
                    m2t = m2.tile([P, 2], F32, tag="m2t")
                    nc.vector.tensor_scalar(out=m2t[:], in0=stg[:, 2:4],
                                            scalar1=1.0 / N, scalar2=None,
                                            op0=ALU.mult)
                    var = m2.tile([P, 2], F32, tag="var")
                    nc.vector.tensor_tensor(out=var[:], in0=mu[:], in1=mu[:],
                                            op=ALU.mult)
                    nc.vector.tensor_tensor(out=var[:], in0=m2t[:], in1=var[:],
                                            op=ALU.subtract)
                    nc.vector.tensor_scalar(out=var[:], in0=var[:], scalar1=EPS,
                                            scalar2=None, op0=ALU.add)
                    std = m2.tile([P, 2], F32, tag="std")
                    nc.scalar.activation(std[:], var[:], ACTF.Sqrt)
                    rstd = m2.tile([P, 2], F32, tag="rstd")
                    nc.vector.reciprocal(rstd[:], std[:])
                    gp = m2.tile([P, 2], F32, tag="gp")
                    nc.vector.tensor_tensor(out=gp[:], in0=gb[:, 0:2], in1=rstd[:],
                                            op=ALU.mult)
                    # bp = beta - mu * gp
                    bp = m2.tile([P, 2], F32, tag="bp")
                    nc.vector.tensor_tensor(out=bp[:], in0=mu[:], in1=gp[:],
                                            op=ALU.mult)
                    nc.vector.tensor_tensor(out=bp[:], in0=gb[:, 2:4], in1=bp[:],
                                            op=ALU.subtract)
                    wpp1 = m2.tile([P, NCLS], F32, tag="wpp1")
                    nc.vector.tensor_scalar_mul(wpp1[:], wm2c1f[:], gp[:, 0:1])
                    wpp2 = m2.tile([MLP_H - P, NCLS], F32, tag="wpp2")
                    nc.vector.tensor_scalar_mul(wpp2[:], wm2c2f[:],
                                                gp[:MLP_H - P, 1:2])
                    cp = m2p.tile([1, NCLS], F32, tag="cp")
                    nc.tensor.matmul(cp[:], lhsT=bp[:, 0:1], rhs=wm2c1f[:],
                                     start=True, stop=False)
                    nc.tensor.matmul(cp[:], lhsT=bp[:MLP_H - P, 1:2], rhs=wm2c2f[:],
                                     start=False, stop=True)
                    cps = m2.tile([1, NCLS], F32, tag="cps")
                    nc.vector.tensor_tensor(out=cps[:], in0=cp[:], in1=bm2[:],
                                            op=ALU.add)
                    nc.sync.dma_start(out=ccc[:], in_=cps[:])
                    cpsb = m2.tile([P, NCLS], F32, tag="cpsb")
                    ccc_row = ccc[0:1, :]
                    nc.gpsimd.dma_start(out=cpsb[:], in_=bass.AP(
                        tensor=ccc_row.tensor, offset=ccc_row.offset,
                        ap=[[0, P]] + ccc_row.ap[1:]))

                    # pass B: out = zT'' @ wpp + c''
                    for j in range(NCHUNK):
                        r0 = j * P
                        rows = min(P, NSHARD - r0)
                        zt1 = m2p.tile([P, P], F32, tag="zt1")
                        nc.tensor.matmul(zt1[:], lhsT=wm1b[:, 0:P],
                                         rhs=x3aug[:, j, :], start=True, stop=True)
                        zt2 = m2p.tile([MLP_H - P, P], F32, tag="zt2")
                        nc.tensor.matmul(zt2[:], lhsT=wm1b[:, P:MLP_H],
                                         rhs=x3aug[:, j, :], start=True, stop=True)
                        zs1 = m2.tile([P, P], F32, tag="zs1")
                        nc.scalar.activation(zs1[:], zt1[:], ACTF.Relu)
                        zs2 = m2.tile([MLP_H - P, P], F32, tag="zs2")
                        nc.scalar.activation(zs2[:], zt2[:], ACTF.Relu)
                        op_ = m2p.tile([P, NCLS], F32, tag="op")
                        nc.tensor.matmul(op_[:], lhsT=zs1[:], rhs=wpp1[:],
                                         start=True, stop=False)
                        nc.tensor.matmul(op_[:], lhsT=zs2[:], rhs=wpp2[:],
                                         start=False, stop=True)
                        ofin = m2.tile([P, NCLS], F32, tag="ofin")
                        nc.vector.tensor_tensor(out=ofin[:rows], in0=op_[:rows],
                                                in1=cpsb[:rows], op=ALU.add)
                        nc.sync.dma_start(out=out[r0:r0 + rows, :], in_=ofin[:rows])

            for _rep in range(reps):
                _run_once(_rep)

        except _SkipRest:
            pass
    nc.finalize()
    return nc


# ----------------------------------------------------------------------------
# Host entry
# ----------------------------------------------------------------------------

def prep_inputs(inputs):
    """Returns (plan, in_maps)."""
    from concourse import mybir
    f8 = mybir.dt.np(mybir.dt.float8e4)
    f32 = np.float32

    Wy1, Wer1, Minv1 = _fold_weights(
        np.asarray(inputs["W1"], f32), np.asarray(inputs["al1"], f32),
        np.asarray(inputs["ar1"], f32))
    Wy2, Wer2, Minv2 = _fold_weights(
        np.asarray(inputs["W2"], f32), np.asarray(inputs["al2"], f32),
        np.asarray(inputs["ar2"], f32))
    plan, earrays = _prep_edges(inputs["src"], inputs["dst"], f8)

    features = np.asarray(inputs["features"], f32)
    featT = np.ascontiguousarray(features.T.astype(np.float16))

    # bias tile [D, 2H]: col lay*4+h = b[lay][h, :], pre-scaled by 0.25
    bias = np.zeros((D, 2 * H), f32)
    b1 = np.asarray(inputs["b1"], f32).reshape(H, D) * 0.25
    b2 = np.asarray(inputs["b2"], f32).reshape(H, D) * 0.25
    for h in range(H):
        bias[:, h] = b1[h]
        bias[:, H + h] = b2[h]

    wm1b = np.zeros((D + 1, MLP_H), f32)
    wm1b[0:D] = np.asarray(inputs["Wm1"], f32)
    wm1b[D] = np.asarray(inputs["bm1"], f32)

    gamma = np.asarray(inputs["gamma"], f32)
    beta = np.asarray(inputs["beta"], f32)
    gbarr = np.zeros((P, 4), f32)
    gbarr[:, 0] = gamma[0:P]
    gbarr[0:MLP_H - P, 1] = gamma[P:MLP_H]
    gbarr[:, 2] = beta[0:P]
    gbarr[0:MLP_H - P, 3] = beta[P:MLP_H]

    consts = {
        "IDENT": np.eye(P, dtype=np.float16),
        "WY1": Wy1, "WER1": Wer1, "WY2": Wy2, "WER2": Wer2,
        "MINV1": Minv1, "MINV2": Minv2,
        "BIAS": bias, "WM1B": wm1b, "GB": gbarr,
        "WM2C1F": np.asarray(inputs["Wm2"], f32)[0:P, :],
        "WM2C2F": np.asarray(inputs["Wm2"], f32)[P:MLP_H, :],
        "BM2": np.asarray(inputs["bm2"], f32).reshape(1, NCLS),
        "ONESF": np.ones((P, 1), f32),
        "ONESH": np.ones((P, 1), np.float16),
    }

    in_maps = []
    for c in range(NCORES):
        IDXa, M8a, MT8a = earrays[c]
        m = dict(consts)
        m["featT"] = featT
        m["fownT"] = np.ascontiguousarray(featT[:, c * NSHARD:(c + 1) * NSHARD])
        m["IDX"] = IDXa
        m["M16"] = M8a
        m["MT8"] = MT8a
        in_maps.append(m)
    return plan, in_maps


def kernel(**inputs):
    from concourse.bass_utils import run_bass_kernel_spmd
    plan, in_maps = prep_inputs(inputs)
    nc = build_nc(plan)
    res = run_bass_kernel_spmd(nc, in_maps, core_ids=list(range(NCORES)))
    out = np.concatenate([res.results[c]["out"] for c in range(NCORES)], axis=0)
    return out.astype(np.float32)


if __name__ == "__main__":
    import time
    t0 = time.time()
    rng = np.random.default_rng(0)
    from concourse import mybir
    plan, _ = _prep_edges(
        np.concatenate([rng.integers(0, N, 800000), np.arange(N)]).astype(np.int32),
        np.concatenate([rng.integers(0, N, 800000), np.arange(N)]).astype(np.int32),
        mybir.dt.np(mybir.dt.float8e4))
    print("edge prep:", time.time() - t0, "totT:", plan["totT"])
    t0 = time.time()
    nc = build_nc(plan)
    print("build:", time.time() - t0)
